# revision 1
# baseline (speedup 1.0000x reference)
"""Contextual loss (CX) kernel for Trainium2, 8 NeuronCores.

Problem: images/gt [1, 256, 96, 96] f32.
  mean_t = mean(gt, axis=(0,2,3))
  i_c, t_c = images - mean_t, gt - mean_t ; L2-normalize along channels
  dot[r, s] = <i_n[:, r], t_n[:, s]>          (r, s over 9216 positions)
  d = clip((1-dot)/2, 0); rel = d / (min_s d + 1e-5)
  w = exp((1-rel)/0.5); cx = w / sum_s w
  loss = -log(mean_s(max_r cx))

Sharding: row-parallel over the 9216 query positions (1152 rows/core).
Each core computes its local column-max of cx -> [128, 9216] (128 SBUF
partitions still to be max-reduced); host does the final max/mean/-log.

Key algebraic identity used on-device: with m = min_s d + eps and
d = max(0, (1-dot)/2), row-min of d equals max(0, (1-rowmax(dot))/2),
and  cx = min(exp((dot-1)/m), 1) / Z  with Z = sum_s exp((dot-1)/m)
(the clamp at 1 never binds for this data: max dot ~ 0.3).
"""

import os
from contextlib import ExitStack

import numpy as np

import concourse.bacc as bacc
import concourse.bass as bass
import concourse.tile as tile
from concourse import masks, mybir
from concourse.bass_utils import run_bass_kernel_spmd

N_CORES = 8
C = 256          # channels
S = 9216         # 96*96 positions
R = S // N_CORES # 1152 query rows per core
P = 128
HALF = S // 2    # 4608
GRP = 1536       # PSUM copy group: 3 banks
NGRP = S // GRP  # 6
EPS_REL = 1e-5

F32 = mybir.dt.float32
BF16 = mybir.dt.bfloat16
AF = mybir.ActivationFunctionType
ALU = mybir.AluOpType


def _build():
    nc = bacc.Bacc(None, target_bir_lowering=False, debug=False)
    gt_d = nc.declare_dram_parameter("gt", [C, S], BF16, isOutput=False)
    img_d = nc.declare_dram_parameter("img", [C, R], BF16, isOutput=False)
    out_d = nc.declare_dram_parameter("acc", [P, S], BF16, isOutput=True)
    # inverse-norm rows staged via DRAM for the partition-broadcast DMA:
    # rows 0..71 = beta (t positions), rows 72..80 = alpha (i positions)
    norm_dram = nc.dram_tensor("norm_scratch", [96, P], BF16)
    NT = S // P   # 72 t-norm tiles
    NI = R // P   # 9 i-norm tiles

    with ExitStack() as ctx:
        tc = ctx.enter_context(tile.TileContext(nc))
        big = ctx.enter_context(tc.tile_pool(name="big", bufs=2))
        wpool = ctx.enter_context(tc.tile_pool(name="wp", bufs=2))
        tnp = ctx.enter_context(tc.tile_pool(name="tnp", bufs=1))
        ipp = ctx.enter_context(tc.tile_pool(name="ipp", bufs=1))
        scr = ctx.enter_context(tc.tile_pool(name="scr", bufs=1))
        accp = ctx.enter_context(tc.tile_pool(name="accp", bufs=1))
        rows = ctx.enter_context(tc.tile_pool(name="rows", bufs=1))
        small = ctx.enter_context(tc.tile_pool(name="small", bufs=6))
        psmm = ctx.enter_context(
            tc.tile_pool(name="psmm", bufs=2, space=bass.MemorySpace.PSUM)
        )
        psn = ctx.enter_context(
            tc.tile_pool(name="psn", bufs=1, space=bass.MemorySpace.PSUM)
        )

        ones_k = rows.tile([P, 1], BF16, tag="ones_k")
        nc.vector.memset(ones_k, 1.0)
        ident = rows.tile([P, P], BF16, tag="ident")
        masks.make_identity(nc, ident[:, :])

        acc = accp.tile([P, S], BF16, tag="acc")
        nc.vector.memset(acc, 0.0)

        # ---------------- prefix: load + center + normalize ----------------
        img_t = []
        for k in range(2):
            im = ipp.tile([P, R], BF16, tag=f"img{k}")
            nc.sync.dma_start(out=im, in_=img_d[k * P : (k + 1) * P, :])
            img_t.append(im)
        gt_t = []
        mu = []
        for k in range(2):
            g = big.tile([P, S], BF16, tag="big")
            nc.sync.dma_start(out=g, in_=gt_d[k * P : (k + 1) * P, :])
            musum = small.tile([P, 1], F32, tag="musum")
            if k == 0:
                # channel sum via ACT accumulator (in-place copy) so the two
                # k-tiles' reductions run on different engines in parallel
                nc.scalar.activation(g, g, AF.Copy, accum_out=musum)
            else:
                nc.vector.tensor_reduce(
                    musum, g, axis=mybir.AxisListType.X, op=ALU.add
                )
            mu_k = small.tile([P, 1], F32, tag="mu")
            nc.vector.tensor_scalar(mu_k, musum, 1.0 / S, None, op0=ALU.mult)
            # center in place
            nc.vector.tensor_scalar(g, g, mu_k, None, op0=ALU.subtract)
            gt_t.append(g)
            mu.append(mu_k)
            nc.vector.tensor_scalar(img_t[k], img_t[k], mu_k, None, op0=ALU.subtract)

        # Squared norms per position, TRANSPOSED: norms_T[p, j] = nrm2 of
        # position j*128+p, via N=1 matmuls (lhsT = squares tile, rhs = ones).
        # All 81 norm columns live in ONE psum bank -> a single batched
        # Ln + Exp gives 1/sqrt with no act-table thrashing.
        ntile = psn.tile([P, 96], F32, tag="normT")
        for h in range(2):  # halves of S to bound scratch
            sqb = scr.tile([P, 2, HALF], BF16, tag="scr")
            for k in range(2):
                # gt squares on DVE (TT mult, 2x) — ACT is busy with i squares
                hs = slice(h * HALF, (h + 1) * HALF)
                nc.vector.tensor_tensor(
                    sqb[:, k, :], gt_t[k][:, hs], gt_t[k][:, hs], op=ALU.mult
                )
            for j in range(NT // 2):
                jj = h * (NT // 2) + j
                sl = slice(j * P, (j + 1) * P)
                nc.tensor.matmul(
                    ntile[:, jj : jj + 1], sqb[:, 0, sl], ones_k, start=True, stop=False
                )
                nc.tensor.matmul(
                    ntile[:, jj : jj + 1], sqb[:, 1, sl], ones_k, start=False, stop=True
                )
        sqi = scr.tile([P, 2, R], BF16, tag="scri")
        for k in range(2):
            nc.scalar.activation(sqi[:, k, :], img_t[k], AF.Square)
        for j in range(NI):
            jj = NT + j
            sl = slice(j * P, (j + 1) * P)
            nc.tensor.matmul(
                ntile[:, jj : jj + 1], sqi[:, 0, sl], ones_k, start=True, stop=False
            )
            nc.tensor.matmul(
                ntile[:, jj : jj + 1], sqi[:, 1, sl], ones_k, start=False, stop=True
            )
        # beta/alpha = exp(-0.5*ln(nrm2)) = 1/sqrt(nrm2)  (Rsqrt is banned)
        nc.scalar.activation(ntile[:, : NT + NI], ntile[:, : NT + NI], AF.Ln)
        ninv = rows.tile([P, 96], BF16, tag="ninv")
        nc.vector.memset(ninv, 0.0)
        nc.scalar.activation(ninv[:, : NT + NI], ntile[:, : NT + NI], AF.Exp, scale=-0.5)
        # transpose [128, 96] -> [96, 128] and stage s-major in DRAM
        ntr = psn.tile([96, P], BF16, tag="ntr")
        nc.tensor.transpose(ntr, ninv, ident)
        ntr_sb = rows.tile([96, P], BF16, tag="ntr_sb")
        nc.scalar.activation(ntr_sb, ntr, AF.Copy)
        nc.sync.dma_start(out=norm_dram[: NT + NI, :], in_=ntr_sb[: NT + NI, :])

        nbase = norm_dram[0:1, 0:1]
        beta_bc = wpool.tile([P, S], BF16, tag="wp")
        t_n0 = tnp.tile([P, S], BF16, tag="tn0")
        t_n1 = tnp.tile([P, S], BF16, tag="tn1")
        t_n = [t_n0, t_n1]
        # broadcast + normalize in halves so stripe-0 matmuls on the first
        # half of t_n can start before the second half is built
        for h in range(2):
            hs = slice(h * HALF, (h + 1) * HALF)
            nc.sync.dma_start(
                out=beta_bc[:, hs],
                in_=bass.AP(
                    tensor=nbase.tensor, offset=h * HALF, ap=[[0, P], [1, HALF]]
                ),
            )
            for k in range(2):
                nc.vector.tensor_tensor(
                    t_n[k][:, hs], gt_t[k][:, hs], beta_bc[:, hs], op=ALU.mult
                )

        abase = norm_dram[NT : NT + 1, 0:1]
        alpha_bc = ipp.tile([P, R], BF16, tag="alpha_bc")
        nc.sync.dma_start(
            out=alpha_bc,
            in_=bass.AP(tensor=abase.tensor, offset=abase.offset, ap=[[0, P], [1, R]]),
        )
        i_n = []
        for k in range(2):
            t = ipp.tile([P, R], BF16, tag=f"in{k}")
            nc.vector.tensor_tensor(t, img_t[k], alpha_bc, op=ALU.mult)
            i_n.append(t)

        # ---------------- main loop: 9 row stripes ----------------
        for si in range(R // P):
            rs = slice(si * P, (si + 1) * P)
            dot = big.tile([P, S], BF16, tag="big")
            run = scr.tile([P, GRP], BF16, tag="run")
            for g in range(NGRP):
                ps = psmm.tile([P, GRP], F32, tag="mm")
                for j3 in range(GRP // 512):
                    off = g * GRP + j3 * 512
                    psl = slice(j3 * 512, (j3 + 1) * 512)
                    nc.tensor.matmul(
                        ps[:, psl], i_n[0][:, rs], t_n[0][:, off : off + 512],
                        start=True, stop=False,
                    )
                    nc.tensor.matmul(
                        ps[:, psl], i_n[1][:, rs], t_n[1][:, off : off + 512],
                        start=False, stop=True,
                    )
                gs = slice(g * GRP, (g + 1) * GRP)
                if g == NGRP - 1:
                    # last group's PSUM evacuation on DVE for engine balance
                    nc.vector.tensor_copy(dot[:, gs], ps)
                else:
                    nc.scalar.activation(dot[:, gs], ps, AF.Copy)
                # running row-max folds in as copies land, so EXP's scale is
                # ready almost immediately after the last copy
                if g == 1:
                    nc.vector.tensor_tensor(
                        run, dot[:, 0:GRP], dot[:, gs], op=ALU.max
                    )
                elif g > 1:
                    nc.vector.tensor_tensor(run, run, dot[:, gs], op=ALU.max)
            rm = small.tile([P, 1], F32, tag="rm")
            nc.vector.tensor_reduce(rm, run, axis=mybir.AxisListType.X, op=ALU.max)

            # m = max(0, (1-rowmax)/2) + eps ; invm = 1/m
            t1 = small.tile([P, 1], F32, tag="t1")
            nc.vector.tensor_scalar(t1, rm, -0.5, 0.5, op0=ALU.mult, op1=ALU.add)
            t2 = small.tile([P, 1], F32, tag="t2")
            nc.vector.tensor_scalar(t2, t1, 0.0, EPS_REL, op0=ALU.max, op1=ALU.add)
            invm = small.tile([P, 1], F32, tag="invm")
            nc.vector.reciprocal(invm, t2)
            nim = small.tile([P, 1], F32, tag="nim")
            nc.vector.tensor_scalar(nim, invm, -1.0, None, op0=ALU.mult)

            # w = exp(dot*invm - invm), Z = row sum of w
            w = wpool.tile([P, S], BF16, tag="wp")
            zsum = small.tile([P, 1], F32, tag="zsum")
            nc.scalar.activation(
                w, dot, AF.Exp, bias=nim, scale=invm, accum_out=zsum
            )
            invz = small.tile([P, 1], F32, tag="invz")
            nc.vector.reciprocal(invz, zsum)

            # acc = max(acc, w * invz). Two ops beat the fused
            # scalar_tensor_tensor: ts runs 4x, tt 2x, stt only 1x.
            if si == R // P - 1:
                # final stripe: work in halves so the output DMA starts early
                for h in range(2):
                    hs = slice(h * HALF, (h + 1) * HALF)
                    nc.vector.tensor_scalar(w[:, hs], w[:, hs], invz, None, op0=ALU.mult)
                    nc.vector.tensor_tensor(acc[:, hs], acc[:, hs], w[:, hs], op=ALU.max)
                    nc.sync.dma_start(out=out_d[:, hs], in_=acc[:, hs])
            else:
                nc.vector.tensor_scalar(w, w, invz, None, op0=ALU.mult)
                nc.vector.tensor_tensor(acc, acc, w, op=ALU.max)

    nc.compile()
    return nc


_NC_CACHE = None


def kernel(images: np.ndarray, gt: np.ndarray) -> np.ndarray:
    global _NC_CACHE
    import ml_dtypes

    img2d = np.ascontiguousarray(
        np.asarray(images, dtype=np.float32).reshape(C, S)
    ).astype(ml_dtypes.bfloat16)
    gt2d = np.ascontiguousarray(
        np.asarray(gt, dtype=np.float32).reshape(C, S)
    ).astype(ml_dtypes.bfloat16)

    if _NC_CACHE is None:
        _NC_CACHE = _build()
    nc = _NC_CACHE

    in_maps = [
        {"gt": gt2d, "img": np.ascontiguousarray(img2d[:, d * R : (d + 1) * R])}
        for d in range(N_CORES)
    ]
    trace = bool(int(os.environ.get("CX_TRACE", "0")))
    res = run_bass_kernel_spmd(nc, in_maps, list(range(N_CORES)), trace=trace)
    kernel.LAST_EXEC_NS = res.exec_time_ns

    # host-side gather: global column max over all 8*128 row groups
    parts = np.stack(
        [np.asarray(res.results[d]["acc"]).astype(np.float32) for d in range(N_CORES)]
    )  # [8, 128, S]
    colmax = parts.max(axis=(0, 1))  # [S]
    cs = colmax.mean()
    loss = -np.log(cs)
    return np.float32(loss)


kernel.LAST_EXEC_NS = None

